# revision 18
# baseline (speedup 1.0000x reference)
"""ConcatenatedLoRALinearSidecarLayer kernel for 8x TRN2 NeuronCores.

Reference computation (per LoRA branch n, then concat over n on the last dim):
    h_n = x @ down_n.T                      # [M, R]
    y_n = (h_n @ up_n.T + bias_n) * (WEIGHT * scales_n)
    out = concat_n(y_n)                     # [M, N*O]

Strategy (v4 -- DMA-roofline oriented, software-pipelined):
  - Data-parallel over tokens M = B*S = 16384 -> 2048 tokens per core.
  - ALL device IO in bf16: x in, down/up weights in, y out. The fp32 result
    is reconstructed on the host (bf16 -> fp32 upcast + bias add). Per core
    ~17MB x-in + ~50MB y-out + ~6MB weights ~= 73MB @ ~420 GB/s ~= 175us,
    which is the binding roofline (PE compute is ~164us).
  - bias*weight*scale folded on the host and added on the HOST: a
    PSUM-sourced tensor_tensor add runs at 1x DVE mode (would cost
    ~250us/core). Device only does PSUM->SBUF copy/cast.
  - Software pipeline: phase 2 of block tb-1 is interleaved between the
    phase-1 dc-groups of block tb, so PSUM->SBUF copy work (the slowest
    per-block burst) is spread uniformly instead of bursting after each
    phase-1; copies alternate DVE / ScalarE.
  - Phase-2 PSUM tiles span 2 banks (2 matmuls each); one FD=1024 copy per
    tile amortizes the per-op overhead (120cyc DVE / ~270cyc ACT).
  - Startup: dT is loaded in 4 quarters interleaved with block-0 x quarter
    loads so the first matmul waits on ~1.8MB, not 22MB; uT (first needed
    ~30us in) is queued after block-0's x.
  - y writes: 24KB-contiguous rows, split per 128-token group into 2x1.5MB
    (last group 4x0.75MB) issued as soon as their chunks are cast.

Wait-slot legalization: this container's walrus accepts at most 1 sync-wait
per instruction; a JSON post-pass splits excess waits onto same-engine NoOps
with identical blocking semantics.
"""

from contextlib import ExitStack

import numpy as np

import concourse.bass as bass
import concourse.mybir as mybir
import concourse.tile as tile

WEIGHT = 0.8
N_CORES = 8
B, S, D = 4, 4096, 4096
NL, R, O = 3, 128, 4096
M = B * S                    # 16384 tokens total
T = M // N_CORES             # 2048 tokens per core
NR = NL * R                  # 384
NO = NL * O                  # 12288

P = 128                      # SBUF partitions
TB = 512                     # token block (phase-1 moving free dim)
DO = D // P                  # 32 contraction chunks
DH = DO // 2                 # d-chunks per x half-load
DQ = DO // 4                 # d-chunks per startup quarter-load
OC = 512                     # phase-2 matmul free dim (fp32 PSUM bank)
OP = 2 * OC                  # phase-2 copy width (2-bank PSUM tile)

F32 = mybir.dt.float32
BF16 = mybir.dt.bfloat16

MAX_WAITS = 1


def build_nc(t_core: int = T) -> bass.Bass:
    assert t_core % TB == 0
    n_tb = t_core // TB

    nc = bass.Bass("TRN2", target_bir_lowering=False, debug=False)

    xT = nc.dram_tensor("xT", [P, n_tb, 2, DH, TB], BF16, kind="ExternalInput")
    dT = nc.dram_tensor("dT", [P, DO, NR], BF16, kind="ExternalInput")
    uT = nc.dram_tensor("uT", [R, NO], BF16, kind="ExternalInput")
    y = nc.dram_tensor("y", [t_core, NO], BF16, kind="ExternalOutput")

    copy_fn = mybir.ActivationFunctionType.Copy

    with tile.TileContext(nc) as tc, ExitStack() as ctx:
        const = ctx.enter_context(tc.tile_pool(name="const", bufs=1))
        xpool = ctx.enter_context(tc.tile_pool(name="xpool", bufs=3))
        hpool = ctx.enter_context(tc.tile_pool(name="hpool", bufs=2))
        ypool = ctx.enter_context(tc.tile_pool(name="ypool", bufs=8))
        ps_h = ctx.enter_context(tc.tile_pool(name="ps_h", bufs=2, space="PSUM"))
        ps_y = ctx.enter_context(tc.tile_pool(name="ps_y", bufs=3, space="PSUM"))

        dT_sb = const.tile([P, DO, NR], BF16, name="dT_sb")
        uT_sb = const.tile([P, NO], BF16, name="uT_sb")

        x_tiles = {}
        for tb in range(n_tb):
            for h in range(2):
                x_tiles[(tb, h)] = xpool.tile(
                    [P, DH, TB], BF16, tag="xt", name=f"xt{tb}_{h}"
                )

        # Startup sequencing: the SP HWDGE ring drains roughly in issue
        # order, so queue pieces in the order compute needs them: block-0 x
        # and dT quarters first, then block-1 x and uT quarters interleaved.
        def dT_q(q):
            nc.gpsimd.dma_start(dT_sb[:, q * DQ:(q + 1) * DQ, :],
                                dT[:, q * DQ:(q + 1) * DQ, :])

        def x_q(tb, h, q):
            nc.gpsimd.dma_start(
                x_tiles[(tb, h)][:, q * DQ:(q + 1) * DQ, :],
                xT[:, tb, h, q * DQ:(q + 1) * DQ, :],
            )

        def uT_q(q):
            nc.gpsimd.dma_start(uT_sb[:, q * (NO // 4):(q + 1) * (NO // 4)],
                                uT[:, q * (NO // 4):(q + 1) * (NO // 4)])

        dT_q(0)
        x_q(0, 0, 0)
        dT_q(1)
        x_q(0, 0, 1)
        dT_q(2)
        x_q(0, 1, 0)
        dT_q(3)
        x_q(0, 1, 1)
        if n_tb > 1:
            x_q(1, 0, 0)
            x_q(1, 0, 1)
            uT_q(0)
            uT_q(1)
            x_q(1, 1, 0)
            uT_q(2)
            uT_q(3)
            x_q(1, 1, 1)
        else:
            for q in range(4):
                uT_q(q)

        HNO = NO // 2

        def emit_p2_group(t0, th, hT, last):
            """Phase 2 for the 128-token sub-block at absolute token t0.

            The ysb staging buffer is two half-row tiles so each half's SBUF
            slot recycles as soon as its own y DMA lands (finer-grained than
            one full-row tile held until the last DMA).
            """
            halves = [
                ypool.tile([P, HNO], BF16, tag="ysb", name=f"ysb{t0}_{hf}")
                for hf in range(2)
            ]
            k = 0
            for n in range(NL):
                lhs = hT[:, n, th * P:(th + 1) * P]
                o0 = n * O
                for op in range(O // OP):
                    yps = ps_y.tile([P, OP], F32, tag="yps",
                                    name=f"yps{t0}_{n}_{op}")
                    for hf in range(2):
                        oc = op * 2 + hf
                        nc.tensor.matmul(
                            yps[:, hf * OC:(hf + 1) * OC],
                            lhs,
                            uT_sb[:, o0 + oc * OC: o0 + (oc + 1) * OC],
                            start=True,
                            stop=True,
                        )
                    c0 = o0 + op * OP        # global column of this tile
                    ysb = halves[c0 // HNO]
                    dst = ysb[:, c0 % HNO: c0 % HNO + OP]
                    if k % 2 == 0:
                        nc.vector.tensor_copy(dst, yps[:])
                    else:
                        nc.scalar.activation(dst, yps[:], copy_fn)
                    k += 1
                    if k in (3, 9) and last:
                        c = k * OP
                        h0 = (c - 3 * OP) // HNO
                        s0 = (c - 3 * OP) % HNO
                        nc.sync.dma_start(
                            y[t0:t0 + P, c - 3 * OP:c],
                            halves[h0][:, s0:s0 + 3 * OP],
                        )
                    if k in (6, 12):
                        hf = k // 6 - 1
                        if last:
                            c = k * OP
                            nc.sync.dma_start(
                                y[t0:t0 + P, c - 3 * OP:c],
                                halves[hf][:, HNO - 3 * OP:],
                            )
                        else:
                            nc.sync.dma_start(
                                y[t0:t0 + P, hf * HNO:(hf + 1) * HNO],
                                halves[hf][:],
                            )

        # Compute passes: full 512-token blocks, except the LAST block which
        # splits into two 256-token half-passes so the un-overlapped epilogue
        # (phase 2 with no phase 1 left to interleave) shrinks to 2 groups.
        passes = [(tb, 0, TB) for tb in range(n_tb - 1)]
        passes += [(n_tb - 1, 0, TB // 2), (n_tb - 1, TB // 2, TB // 2)]

        prev = None  # (tb, off, ln, hT) of the pass whose phase 2 is pending
        for pi, (tb, off, ln) in enumerate(passes):
            xts = [x_tiles[(tb, 0)], x_tiles[(tb, 1)]]
            # Prefetch the NEXT block's x ahead of this block's y-write
            # DMAs on the (FIFO) SP queue. (Blocks 0-1 load in the preamble.)
            if off == 0 and 2 <= tb + 1 < n_tb:
                for h in range(2):
                    nc.gpsimd.dma_start(
                        x_tiles[(tb + 1, h)][:], xT[:, tb + 1, h, :, :]
                    )

            # Phase-1 matmuls, branch-outer so only ONE hps bank accumulates
            # at a time (ps_h bufs=2 covers copy/accumulate overlap, freeing
            # PSUM banks for ps_y depth 3). hT copy for branch n (ScalarE)
            # is emitted right after that branch's last accumulation.
            hT = hpool.tile([P, NL, TB], BF16, tag="hT", name=f"hT{tb}_{off}")
            p1_seq = []
            for n in range(NL):
                hp = ps_h.tile([P, TB], F32, tag="hps", name=f"hps{tb}_{off}_{n}")
                for dc in range(DO):
                    p1_seq.append((n, dc, hp))

            n_prev_g = (prev[2] // P) if prev is not None else 0
            chunk = 96 // n_prev_g if n_prev_g else 96
            gi = 0
            for idx, (n, dc, hp) in enumerate(p1_seq):
                xs = xts[dc // DH][:, dc % DH, off:off + ln]
                nc.tensor.matmul(
                    hp[:, :ln],
                    dT_sb[:, dc, n * R:(n + 1) * R],
                    xs,
                    start=(dc == 0),
                    stop=(dc == DO - 1),
                )
                if dc == DO - 1:
                    nc.scalar.activation(hT[:, n, :ln], hp[:, :ln], copy_fn)
                if prev is not None and (idx + 1) % chunk == 0 and gi < n_prev_g:
                    ptb, poff, _, phT = prev
                    emit_p2_group(ptb * TB + poff + gi * P, gi, phT, False)
                    gi += 1
            prev = (tb, off, ln, hT)

        # Epilogue: phase 2 of the final pass.
        ptb, poff, pln, phT = prev
        for gi in range(pln // P):
            emit_p2_group(ptb * TB + poff + gi * P, gi, phT,
                          gi == pln // P - 1)

    _wrap_to_json_with_wait_split(nc)
    return nc


def _legalize_wait_counts(bir: dict) -> None:
    """Split multi-wait instructions: this walrus accepts only ONE sync-wait
    per instruction. Excess waits move onto NoOps inserted just before the
    instruction on the same engine -- identical blocking semantics."""
    n_new = 0
    for fn in bir.get("functions", []):
        for blk in fn.get("blocks", []):
            insts = blk.get("instructions", [])
            out = []
            for inst in insts:
                si = inst.get("sync_info")
                waits = (si or {}).get("on_wait") or []
                if len(waits) > MAX_WAITS:
                    for w in waits[:-1]:
                        nonlocal_name = f"I-waitsplit-{id(inst)}-{n_new}"
                        n_new += 1
                        out.append({
                            "debug": inst.get("debug", 0),
                            "engine": inst["engine"],
                            "ins": [],
                            "name": nonlocal_name,
                            "opcode": "NoOp",
                            "outs": [],
                            "sync_info": {"on_update": [], "on_wait": [w]},
                        })
                    si["on_wait"] = [waits[-1]]
                out.append(inst)
            blk["instructions"] = out


def _wrap_to_json_with_wait_split(nc) -> None:
    import json as _json

    orig = nc.to_json_bytes

    def patched():
        d = _json.loads(orig())
        _legalize_wait_counts(d)
        return _json.dumps(d).encode()

    nc.to_json_bytes = patched


def prep_inputs(x, down, up, bias, scales):
    """Host-side marshalling: transpose + fold scales + bf16 casts.

    Returns (per-core in_maps, bias_w) where bias_w is the host-side
    fp32 bias (already scaled) to add after the device run.
    """
    import ml_dtypes

    x = np.asarray(x, dtype=np.float32)
    down = np.asarray(down, dtype=np.float32)
    up = np.asarray(up, dtype=np.float32)
    bias = np.asarray(bias, dtype=np.float32)
    scales = np.asarray(scales, dtype=np.float32)

    ws = (WEIGHT * scales).astype(np.float32)                       # [NL]
    bias_w = (bias * ws[:, None]).reshape(NO).astype(np.float32)    # [NO]

    # down [NL,R,D] -> [D, NR] -> [do, di, NR] -> [di=128, do, NR]
    dTf = np.ascontiguousarray(
        np.transpose(down, (2, 0, 1)).reshape(DO, P, NR).transpose(1, 0, 2)
    ).astype(ml_dtypes.bfloat16)
    # up [NL,O,R] * ws -> [R, NO]
    uTf = np.ascontiguousarray(
        np.transpose(up * ws[:, None, None], (2, 0, 1)).reshape(R, NO)
    ).astype(ml_dtypes.bfloat16)

    # x -> [D, M] -> per-core [di, tb, half, dh, t] fully-contiguous tiles
    xTf = np.ascontiguousarray(x.reshape(M, D).T).astype(ml_dtypes.bfloat16)

    n_tb = T // TB
    in_maps = []
    for c in range(N_CORES):
        xc = xTf[:, c * T:(c + 1) * T]                    # [D, T]
        xc = xc.reshape(2, DH, P, n_tb, TB)               # [half, dh, di, tb, t]
        xc = np.ascontiguousarray(xc.transpose(2, 3, 0, 1, 4))  # [di,tb,h,dh,t]
        in_maps.append({
            "xT": xc,
            "dT": dTf,
            "uT": uTf,
        })
    return in_maps, bias_w


_CACHED_NC = None


def kernel(x, down, up, bias, scales):
    global _CACHED_NC
    from concourse.bass_utils import run_bass_kernel_spmd

    in_maps, bias_w = prep_inputs(x, down, up, bias, scales)
    if _CACHED_NC is None:
        _CACHED_NC = build_nc(T)
    res = run_bass_kernel_spmd(_CACHED_NC, in_maps, core_ids=list(range(N_CORES)))
    out = np.concatenate(
        [np.asarray(r["y"]).astype(np.float32) for r in res.results], axis=0
    )
    out += bias_w[None, :]
    return out.reshape(B, S, NO)


# revision 20
# speedup vs baseline: 1.0239x; 1.0239x over previous
"""ConcatenatedLoRALinearSidecarLayer kernel for 8x TRN2 NeuronCores.

Reference computation (per LoRA branch n, then concat over n on the last dim):
    h_n = x @ down_n.T                      # [M, R]
    y_n = (h_n @ up_n.T + bias_n) * (WEIGHT * scales_n)
    out = concat_n(y_n)                     # [M, N*O]

Strategy (final -- DMA-roofline oriented, software-pipelined):
  - Data-parallel over tokens M = B*S = 16384 -> 2048 tokens per core.
  - ALL device IO in bf16: x in, down/up weights in, y out. The fp32 result
    is reconstructed on the host (bf16 -> fp32 upcast + bias add). Per core
    ~17MB x-in + ~50MB y-out + ~6MB weights ~= 73MB @ ~420 GB/s ~= 175us,
    which is the binding roofline (PE compute is ~164us).
  - bias*weight*scale folded on the host and added on the HOST: a
    PSUM-sourced tensor_tensor add runs at 1x DVE mode (would cost
    ~250us/core). Device only does PSUM->SBUF copy/cast.
  - Software pipeline: phase 2 of block tb-1 is interleaved between the
    phase-1 dc-groups of block tb, so PSUM->SBUF copy work (the slowest
    per-block burst) is spread uniformly instead of bursting after each
    phase-1; copies alternate DVE / ScalarE.
  - Phase-2 PSUM tiles span 2 banks (2 matmuls each); one FD=1024 copy per
    tile amortizes the per-op overhead (120cyc DVE / ~270cyc ACT).
  - Startup: dT is loaded in 4 quarters interleaved with block-0 x quarter
    loads so the first matmul waits on ~1.8MB, not 22MB; uT (first needed
    ~30us in) is queued after block-0's x.
  - y writes: 24KB-contiguous rows, split per 128-token group into 2x1.5MB
    (last group 4x0.75MB) issued as soon as their chunks are cast.

Wait-slot legalization: this container's walrus accepts at most 1 sync-wait
per instruction; a JSON post-pass splits excess waits onto same-engine NoOps
with identical blocking semantics.
"""

from contextlib import ExitStack

import numpy as np

import concourse.bass as bass
import concourse.mybir as mybir
import concourse.tile as tile

WEIGHT = 0.8
N_CORES = 8
B, S, D = 4, 4096, 4096
NL, R, O = 3, 128, 4096
M = B * S                    # 16384 tokens total
T = M // N_CORES             # 2048 tokens per core
NR = NL * R                  # 384
NO = NL * O                  # 12288

P = 128                      # SBUF partitions
TB = 512                     # token block (phase-1 moving free dim)
DO = D // P                  # 32 contraction chunks
DH = DO // 2                 # d-chunks per x half-load
DQ = DO // 4                 # d-chunks per startup quarter-load
OC = 512                     # phase-2 matmul free dim (fp32 PSUM bank)
OP = 2 * OC                  # phase-2 copy width (2-bank PSUM tile)

F32 = mybir.dt.float32
BF16 = mybir.dt.bfloat16

MAX_WAITS = 1


def build_nc(t_core: int = T) -> bass.Bass:
    assert t_core % TB == 0
    n_tb = t_core // TB

    nc = bass.Bass("TRN2", target_bir_lowering=False, debug=False)

    xT = nc.dram_tensor("xT", [P, n_tb, 2, DH, TB], BF16, kind="ExternalInput")
    dT = nc.dram_tensor("dT", [P, DO, NR], BF16, kind="ExternalInput")
    uT = nc.dram_tensor("uT", [R, NO], BF16, kind="ExternalInput")
    y = nc.dram_tensor("y", [t_core, NO], BF16, kind="ExternalOutput")

    copy_fn = mybir.ActivationFunctionType.Copy

    with tile.TileContext(nc) as tc, ExitStack() as ctx:
        const = ctx.enter_context(tc.tile_pool(name="const", bufs=1))
        xpool = ctx.enter_context(tc.tile_pool(name="xpool", bufs=4))
        hpool = ctx.enter_context(tc.tile_pool(name="hpool", bufs=2))
        ypool = ctx.enter_context(tc.tile_pool(name="ypool", bufs=7))
        ps_h = ctx.enter_context(tc.tile_pool(name="ps_h", bufs=2, space="PSUM"))
        ps_y = ctx.enter_context(tc.tile_pool(name="ps_y", bufs=3, space="PSUM"))

        dT_sb = const.tile([P, DO, NR], BF16, name="dT_sb")
        uT_sb = const.tile([P, NO], BF16, name="uT_sb")

        x_tiles = {}
        for tb in range(n_tb):
            for h in range(2):
                x_tiles[(tb, h)] = xpool.tile(
                    [P, DH, TB], BF16, tag="xt", name=f"xt{tb}_{h}"
                )

        # Startup sequencing: the SP HWDGE ring drains roughly in issue
        # order, so queue pieces in the order compute needs them: block-0 x
        # and dT quarters first, then block-1 x and uT quarters interleaved.
        def dT_q(q):
            nc.gpsimd.dma_start(dT_sb[:, q * DQ:(q + 1) * DQ, :],
                                dT[:, q * DQ:(q + 1) * DQ, :])

        def x_q(tb, h, q):
            nc.gpsimd.dma_start(
                x_tiles[(tb, h)][:, q * DQ:(q + 1) * DQ, :],
                xT[:, tb, h, q * DQ:(q + 1) * DQ, :],
            )

        def uT_q(q):
            nc.gpsimd.dma_start(uT_sb[:, q * (NO // 4):(q + 1) * (NO // 4)],
                                uT[:, q * (NO // 4):(q + 1) * (NO // 4)])

        dT_q(0)
        x_q(0, 0, 0)
        dT_q(1)
        x_q(0, 0, 1)
        dT_q(2)
        x_q(0, 1, 0)
        dT_q(3)
        x_q(0, 1, 1)
        if n_tb > 1:
            x_q(1, 0, 0)
            x_q(1, 0, 1)
            uT_q(0)
            uT_q(1)
            x_q(1, 1, 0)
            uT_q(2)
            uT_q(3)
            x_q(1, 1, 1)
        else:
            for q in range(4):
                uT_q(q)

        HNO = NO // 2

        def emit_p2_group(t0, th, hT, last):
            """Phase 2 for the 128-token sub-block at absolute token t0.

            The ysb staging buffer is two half-row tiles so each half's SBUF
            slot recycles as soon as its own y DMA lands (finer-grained than
            one full-row tile held until the last DMA).
            """
            halves = [
                ypool.tile([P, HNO], BF16, tag="ysb", name=f"ysb{t0}_{hf}")
                for hf in range(2)
            ]
            k = 0
            for n in range(NL):
                lhs = hT[:, n, th * P:(th + 1) * P]
                o0 = n * O
                for op in range(O // OP):
                    yps = ps_y.tile([P, OP], F32, tag="yps",
                                    name=f"yps{t0}_{n}_{op}")
                    for hf in range(2):
                        oc = op * 2 + hf
                        nc.tensor.matmul(
                            yps[:, hf * OC:(hf + 1) * OC],
                            lhs,
                            uT_sb[:, o0 + oc * OC: o0 + (oc + 1) * OC],
                            start=True,
                            stop=True,
                        )
                    c0 = o0 + op * OP        # global column of this tile
                    ysb = halves[c0 // HNO]
                    dst = ysb[:, c0 % HNO: c0 % HNO + OP]
                    if k % 2 == 0:
                        nc.vector.tensor_copy(dst, yps[:])
                    else:
                        nc.scalar.activation(dst, yps[:], copy_fn)
                    k += 1
                    if k in (3, 9) and last:
                        c = k * OP
                        h0 = (c - 3 * OP) // HNO
                        s0 = (c - 3 * OP) % HNO
                        nc.sync.dma_start(
                            y[t0:t0 + P, c - 3 * OP:c],
                            halves[h0][:, s0:s0 + 3 * OP],
                        )
                    if k in (6, 12):
                        hf = k // 6 - 1
                        if last:
                            c = k * OP
                            nc.sync.dma_start(
                                y[t0:t0 + P, c - 3 * OP:c],
                                halves[hf][:, HNO - 3 * OP:],
                            )
                        else:
                            nc.sync.dma_start(
                                y[t0:t0 + P, hf * HNO:(hf + 1) * HNO],
                                halves[hf][:],
                            )

        # Compute passes: full 512-token blocks, except the LAST block which
        # splits into two 256-token half-passes so the un-overlapped epilogue
        # (phase 2 with no phase 1 left to interleave) shrinks to 2 groups.
        passes = [(tb, 0, TB) for tb in range(n_tb - 1)]
        passes += [(n_tb - 1, 0, TB // 2), (n_tb - 1, TB // 2, TB // 2)]

        prev = None  # (tb, off, ln, hT) of the pass whose phase 2 is pending
        for pi, (tb, off, ln) in enumerate(passes):
            xts = [x_tiles[(tb, 0)], x_tiles[(tb, 1)]]
            # Prefetch the NEXT block's x ahead of this block's y-write
            # DMAs on the (FIFO) SP queue. (Blocks 0-1 load in the preamble.)
            if off == 0 and 2 <= tb + 1 < n_tb:
                for h in range(2):
                    nc.gpsimd.dma_start(
                        x_tiles[(tb + 1, h)][:], xT[:, tb + 1, h, :, :]
                    )

            # Phase-1 matmuls, branch-outer so only ONE hps bank accumulates
            # at a time (ps_h bufs=2 covers copy/accumulate overlap, freeing
            # PSUM banks for ps_y depth 3). hT copy for branch n (ScalarE)
            # is emitted right after that branch's last accumulation.
            hT = hpool.tile([P, NL, TB], BF16, tag="hT", name=f"hT{tb}_{off}")
            p1_seq = []
            for n in range(NL):
                hp = ps_h.tile([P, TB], F32, tag="hps", name=f"hps{tb}_{off}_{n}")
                for dc in range(DO):
                    p1_seq.append((n, dc, hp))

            n_prev_g = (prev[2] // P) if prev is not None else 0
            chunk = 96 // n_prev_g if n_prev_g else 96
            gi = 0
            for idx, (n, dc, hp) in enumerate(p1_seq):
                xs = xts[dc // DH][:, dc % DH, off:off + ln]
                nc.tensor.matmul(
                    hp[:, :ln],
                    dT_sb[:, dc, n * R:(n + 1) * R],
                    xs,
                    start=(dc == 0),
                    stop=(dc == DO - 1),
                )
                if dc == DO - 1:
                    nc.scalar.activation(hT[:, n, :ln], hp[:, :ln], copy_fn)
                if prev is not None and (idx + 1) % chunk == 0 and gi < n_prev_g:
                    ptb, poff, _, phT = prev
                    emit_p2_group(ptb * TB + poff + gi * P, gi, phT, False)
                    gi += 1
            prev = (tb, off, ln, hT)

        # Epilogue: phase 2 of the final pass.
        ptb, poff, pln, phT = prev
        for gi in range(pln // P):
            emit_p2_group(ptb * TB + poff + gi * P, gi, phT,
                          gi == pln // P - 1)

    _wrap_to_json_with_wait_split(nc)
    return nc


def _legalize_wait_counts(bir: dict) -> None:
    """Split multi-wait instructions: this walrus accepts only ONE sync-wait
    per instruction. Excess waits move onto NoOps inserted just before the
    instruction on the same engine -- identical blocking semantics."""
    n_new = 0
    for fn in bir.get("functions", []):
        for blk in fn.get("blocks", []):
            insts = blk.get("instructions", [])
            out = []
            for inst in insts:
                si = inst.get("sync_info")
                waits = (si or {}).get("on_wait") or []
                if len(waits) > MAX_WAITS:
                    for w in waits[:-1]:
                        nonlocal_name = f"I-waitsplit-{id(inst)}-{n_new}"
                        n_new += 1
                        out.append({
                            "debug": inst.get("debug", 0),
                            "engine": inst["engine"],
                            "ins": [],
                            "name": nonlocal_name,
                            "opcode": "NoOp",
                            "outs": [],
                            "sync_info": {"on_update": [], "on_wait": [w]},
                        })
                    si["on_wait"] = [waits[-1]]
                out.append(inst)
            blk["instructions"] = out


def _wrap_to_json_with_wait_split(nc) -> None:
    import json as _json

    orig = nc.to_json_bytes

    def patched():
        d = _json.loads(orig())
        _legalize_wait_counts(d)
        return _json.dumps(d).encode()

    nc.to_json_bytes = patched


def prep_inputs(x, down, up, bias, scales):
    """Host-side marshalling: transpose + fold scales + bf16 casts.

    Returns (per-core in_maps, bias_w) where bias_w is the host-side
    fp32 bias (already scaled) to add after the device run.
    """
    import ml_dtypes

    x = np.asarray(x, dtype=np.float32)
    down = np.asarray(down, dtype=np.float32)
    up = np.asarray(up, dtype=np.float32)
    bias = np.asarray(bias, dtype=np.float32)
    scales = np.asarray(scales, dtype=np.float32)

    ws = (WEIGHT * scales).astype(np.float32)                       # [NL]
    bias_w = (bias * ws[:, None]).reshape(NO).astype(np.float32)    # [NO]

    # down [NL,R,D] -> [D, NR] -> [do, di, NR] -> [di=128, do, NR]
    dTf = np.ascontiguousarray(
        np.transpose(down, (2, 0, 1)).reshape(DO, P, NR).transpose(1, 0, 2)
    ).astype(ml_dtypes.bfloat16)
    # up [NL,O,R] * ws -> [R, NO]
    uTf = np.ascontiguousarray(
        np.transpose(up * ws[:, None, None], (2, 0, 1)).reshape(R, NO)
    ).astype(ml_dtypes.bfloat16)

    # x -> [D, M] -> per-core [di, tb, half, dh, t] fully-contiguous tiles
    xTf = np.ascontiguousarray(x.reshape(M, D).T).astype(ml_dtypes.bfloat16)

    n_tb = T // TB
    in_maps = []
    for c in range(N_CORES):
        xc = xTf[:, c * T:(c + 1) * T]                    # [D, T]
        xc = xc.reshape(2, DH, P, n_tb, TB)               # [half, dh, di, tb, t]
        xc = np.ascontiguousarray(xc.transpose(2, 3, 0, 1, 4))  # [di,tb,h,dh,t]
        in_maps.append({
            "xT": xc,
            "dT": dTf,
            "uT": uTf,
        })
    return in_maps, bias_w


_CACHED_NC = None


def kernel(x, down, up, bias, scales):
    global _CACHED_NC
    from concourse.bass_utils import run_bass_kernel_spmd

    in_maps, bias_w = prep_inputs(x, down, up, bias, scales)
    if _CACHED_NC is None:
        _CACHED_NC = build_nc(T)
    res = run_bass_kernel_spmd(_CACHED_NC, in_maps, core_ids=list(range(N_CORES)))
    out = np.concatenate(
        [np.asarray(r["y"]).astype(np.float32) for r in res.results], axis=0
    )
    out += bias_w[None, :]
    return out.reshape(B, S, NO)
